# revision 1
# baseline (speedup 1.0000x reference)
"""MinCutNet (2x GCN + dense_mincut_pool losses) as an 8-core Trainium2
Bass/Tile kernel.

Sharding: nodes row-wise across 8 cores (1280 nodes/core, padded N=10240).
GCN scatter (segment_sum) runs as sorted-COO one-hot matmuls on the PE;
per-edge feature gathers use SWDGE dma_gather from core-local HBM copies of
the full activation matrix, which are refreshed between layers with
AllGather collectives. Final scalar terms reduce with a tiny AllReduce.
"""

import os
import sys

sys.path.insert(0, "/opt/trn_rl_repo")

import numpy as np

import concourse.bass as bass
import concourse.mybir as mybir
import concourse.tile as tile
from concourse import library_config
from concourse.bass_utils import run_bass_kernel_spmd
from concourse.library_overlay import lower_extended_insts
from concourse.vector_clock import ScopedClock

# ---------------------------------------------------------------- constants
N, E = 10000, 320000
FIN, FH, K = 128, 256, 64
C = 8               # cores
P = 128             # partitions
NPAD = 10240        # 80 blocks of 128
SHARD = NPAD // C   # 1280 nodes per core
BLK = SHARD // P    # 10 blocks per core
NBLK = NPAD // P    # 80 blocks total
K1 = 0              # split-AG piece sizes; 0 = single AllGather (collectives
K2 = 0              # block the Pool queue, so splitting them stalls the gathers)
F32 = mybir.dt.float32
BF16 = mybir.dt.bfloat16
I16 = mybir.dt.int16
import ml_dtypes

NPBF16 = ml_dtypes.bfloat16

_DEBUG_OUTPUTS = bool(int(os.environ.get("KERNEL_DEBUG_OUTPUTS", "0")))
_MAX_PHASE = int(os.environ.get("KERNEL_MAX_PHASE", "9"))


# ------------------------------------------------------- tile drain patch
def _patched_drain_and_barrier(self, tick_clock, wait_clock):
    """walrus in this container rejects >1 sync-wait command on the tail
    Drain; spread the waits across SP nops (1 wait each)."""
    nc = self.nc
    drain_inst = nc.sync.drain()
    wait_clock.add_sem_waits(
        drain_inst.ins, ScopedClock({None: tick_clock.global_clock})
    )
    waits = list(drain_inst.ins.sync_info.on_wait)
    if len(waits) > 1:
        upd = list(drain_inst.ins.sync_info.on_update)
        drain_inst.ins.sync_info = mybir.SyncInfo(on_wait=waits[:1], on_update=upd)
        for i, w in enumerate(waits[1:]):
            nop = nc.sync.nop(nofuse=True, hint=f"tailwait{i}")
            nop.ins.sync_info = mybir.SyncInfo(on_wait=[w], on_update=[])
    nc.all_engine_barrier()
    assert self.sems is not None
    popped = nc._tile_sem_poison_stack.pop()
    assert popped is self._sem_poison
    nc.clear_and_free_semaphores(list(self.sems.allocated().values()))
    nc.all_engine_barrier()


tile.TileContext._drain_and_barrier = _patched_drain_and_barrier

_noop_ctr = [0]


def _split_excess_waits(nc, lim=1):
    """walrus in this container caps sync-wait commands per instruction;
    spill excess waits onto same-engine NOPs placed just before."""
    nsplit = 0
    for fn in nc.m.functions:
        for b in fn.blocks:
            newl = []
            changed = False
            for inst in b.instructions:
                si = inst.sync_info
                if si is not None and len(si.on_wait) > lim:
                    waits = list(si.on_wait)
                    head, tail = waits[: len(waits) - lim], waits[len(waits) - lim :]
                    for i in range(0, len(head), lim):
                        _noop_ctr[0] += 1
                        nop = mybir.InstNoOp(
                            name=f"waitnop-{_noop_ctr[0]}",
                            sync_info=mybir.SyncInfo(
                                on_wait=head[i : i + lim], on_update=[]
                            ),
                            bass_nofuse=True,
                            engine=inst.engine,
                        )
                        newl.append(nop)
                    inst.sync_info = mybir.SyncInfo(
                        on_wait=tail, on_update=list(si.on_update)
                    )
                    nsplit += 1
                    changed = True
                newl.append(inst)
            if changed:
                b.instructions = newl
    return nsplit


# ------------------------------------------------------- host preprocessing
def _bucket_edges(src, dst, w, ntiles):
    """Partition edges by 128-node dst block; pad each (core, block) bucket
    to ntiles*128 entries. Returns per-core [BLK, T*128] arrays."""
    T = ntiles
    a_src = np.zeros((C, BLK, T * P), np.int16)
    a_dloc = np.zeros((C, BLK, T * P), np.float32)
    a_w = np.zeros((C, BLK, T * P), np.float32)
    blk = dst // P
    order = np.argsort(blk, kind="stable")
    src, dst, w, blk = src[order], dst[order], w[order], blk[order]
    counts = np.bincount(blk, minlength=NBLK)
    starts = np.concatenate([[0], np.cumsum(counts)])
    for b in range(NBLK):
        c, lb = divmod(b, BLK)
        s, e = starts[b], starts[b + 1]
        n = e - s
        a_src[c, lb, :n] = src[s:e]
        a_dloc[c, lb, :n] = (dst[s:e] - b * P).astype(np.float32)
        a_w[c, lb, :n] = w[s:e]
    return a_src, a_dloc, a_w


def _idx_layout(a_src, T):
    """[C, BLK, T*128] int16 -> dma_gather idx tables [C, 128, BLK*T*8]."""
    out = np.zeros((C, P, BLK * T * 8), np.int16)
    for c in range(C):
        for b in range(BLK):
            arr = a_src[c, b]  # [T*128]
            tab = arr.reshape(T * 8, 16).T  # [16, T*8]; idx i -> [i%16, i//16]
            out[c, :, b * T * 8 : (b + 1) * T * 8] = np.tile(tab, (8, 1))
    return out


def _tile_layout(a, T):
    """[C, BLK, T*128] f32 -> [C, 128, BLK*T] with [p, b*T+t] = a[c,b,t*128+p]."""
    return np.ascontiguousarray(
        a.reshape(C, BLK, T, P).transpose(0, 3, 1, 2).reshape(C, P, BLK * T)
    )


def _run_table(dst, w, L, self_loop):
    """Padded per-dst weight runs [NPAD, L]."""
    tab = np.zeros((NPAD, L), np.float32)
    order = np.argsort(dst, kind="stable")
    dsts, ws = dst[order], w[order]
    counts = np.bincount(dsts, minlength=NPAD)
    starts = np.concatenate([[0], np.cumsum(counts)])[:-1]
    pos = np.arange(len(dsts)) - starts[dsts]
    tab[dsts, pos] = ws
    if self_loop:
        tab[np.arange(NPAD), counts] = 1.0
    return tab


def _shard_rows(a):
    """[NPAD, L] -> per-core [C, 128, BLK*L] ([p, b*L+j] = a[c*1280+b*128+p, j])."""
    L = a.shape[1]
    return np.ascontiguousarray(
        a.reshape(C, BLK, P, L).transpose(0, 2, 1, 3).reshape(C, P, BLK * L)
    )


def preprocess(edge_index, edge_weight):
    row = edge_index[0].astype(np.int64)
    col = edge_index[1].astype(np.int64)
    ew = edge_weight.astype(np.float32)

    # GCN message-passing tables (edges + self loops), bucketed by col (dst)
    loops = np.arange(N, dtype=np.int64)
    gsrc = np.concatenate([row, loops])
    gdst = np.concatenate([col, loops])
    gw = np.concatenate([ew, np.ones(N, np.float32)])
    gcnt = np.bincount(gdst // P, minlength=NBLK)
    TG = int(np.ceil(gcnt.max() / P))
    g_src, g_dloc, g_w = _bucket_edges(gsrc, gdst, gw, TG)

    # pool tables: adj@s -> gather col, scatter row (raw edges only)
    pcnt = np.bincount(row // P, minlength=NBLK)
    TP = int(np.ceil(max(pcnt.max(), 1) / P))
    p_src, p_dloc, p_w = _bucket_edges(col.astype(np.int64), row, ew, TP)

    # degree run tables (raw edges; self-loop weight 1 appended per node)
    LC = int(np.bincount(col, minlength=NPAD).max()) + 1  # + self-loop slot
    deg_tab = _run_table(col, ew, LC, self_loop=True)  # pad nodes get deg=1
    LR = max(int(np.bincount(row, minlength=NPAD).max()), 1)
    rowdeg_tab = _run_table(row, ew, LR, self_loop=False)

    mask = np.zeros((NPAD,), np.float32)
    mask[:N] = 1.0

    deg_full = np.ascontiguousarray(
        deg_tab.reshape(NBLK, P, LC).transpose(1, 0, 2).reshape(P, NBLK * LC)
    )

    # Split-AllGather row permutations: piece 1 = first kb blocks of every
    # core's shard (rank-major), piece 2 = the rest.
    def split_rowof(n, kb):
        c, loc = n // SHARD, n % SHARD
        cut = kb * P
        return np.where(
            loc < cut,
            c * cut + loc,
            C * cut + c * (SHARD - cut) + (loc - cut),
        )

    g_src2 = split_rowof(g_src.astype(np.int64), K1).astype(np.int16)
    p_src2 = split_rowof(p_src.astype(np.int64), K2).astype(np.int16)
    tabs = dict(
        TG=TG,
        TP=TP,
        LC=LC,
        LR=LR,
        g_idx=_idx_layout(g_src, TG),
        g_idx2=_idx_layout(g_src2, TG),
        g_dloc=_tile_layout(g_dloc, TG),
        g_w=_tile_layout(g_w, TG),
        p_idx=_idx_layout(p_src2, TP),
        p_dloc=_tile_layout(p_dloc, TP),
        p_w=_tile_layout(p_w, TP),
        deg=_shard_rows(deg_tab).astype(NPBF16),
        deg_full=deg_full.astype(NPBF16),
        rowdeg=_shard_rows(rowdeg_tab),
        mask=_shard_rows(mask[:, None]),  # [C, 128, BLK]
    )
    return tabs


# --------------------------------------------------------- device program
def build_program(TG, TP, LC, LR, for_sim=False):
    nc = bass.Bass(num_devices=C)
    dp = nc.declare_dram_parameter

    x_fl = dp("x_full", [NPAD, FIN], BF16, isOutput=False)
    w1 = dp("W1", [FIN, FH], F32, isOutput=False)
    w2 = dp("W2", [FH, FH], F32, isOutput=False)
    wp = dp("Wp", [FH, K], F32, isOutput=False)
    b1 = dp("b1", [1, FH], F32, isOutput=False)
    b2 = dp("b2", [1, FH], F32, isOutput=False)
    bp = dp("bp", [1, K], F32, isOutput=False)
    g_idx = dp("g_idx", [P, BLK * TG * 8], I16, isOutput=False)
    g_idx2 = dp("g_idx2", [P, BLK * TG * 8], I16, isOutput=False)
    g_dloc = dp("g_dloc", [P, BLK * TG], F32, isOutput=False)
    g_w = dp("g_w", [P, BLK * TG], F32, isOutput=False)
    p_idx = dp("p_idx", [P, BLK * TP * 8], I16, isOutput=False)
    p_dloc = dp("p_dloc", [P, BLK * TP], F32, isOutput=False)
    p_w = dp("p_w", [P, BLK * TP], F32, isOutput=False)
    deg_t = dp("deg", [P, BLK * LC], BF16, isOutput=False)
    degf_t = dp("deg_full", [P, NBLK * LC], BF16, isOutput=False)
    rowdeg_t = dp("rowdeg", [P, BLK * LR], F32, isOutput=False)
    mask_t = dp("mask", [P, BLK], F32, isOutput=False)
    iota_t = dp("iota", [P, P], F32, isOutput=False)
    iotab_t = dp("iotab", [P, P], BF16, isOutput=False)
    ident_t = dp("ident", [P, P], F32, isOutput=False)
    id64_t = dp("id64e", [K, K], F32, isOutput=False)  # I/sqrt(K)
    ones_t = dp("ones", [P, 1], F32, isOutput=False)
    ones_row_t = dp("ones_row", [1, P], F32, isOutput=False)

    out_t = dp("out", [1, 1], F32, isOutput=True)
    dbg = {}
    if _DEBUG_OUTPUTS:
        dbg["y1"] = dp("dbg_y1", [NPAD, FH], BF16, isOutput=True)
        dbg["s"] = dp("dbg_s", [NPAD, K], F32, isOutput=True)
        dbg["numden"] = dp("dbg_numden", [1, 2], F32, isOutput=True)
        dbg["ss"] = dp("dbg_ss", [K, K], F32, isOutput=True)

    # internal DRAM
    xs_full = nc.dram_tensor("xs_full", [NPAD, FIN], BF16)
    y1_in = nc.dram_tensor("y1_in", [SHARD, FH], BF16)
    y1_full = nc.dram_tensor("y1_full", [NPAD, FH], BF16, addr_space="Shared")
    s_in = nc.dram_tensor("s_in", [SHARD, K], F32)
    s_full = nc.dram_tensor("s_full", [NPAD, K], F32, addr_space="Shared")
    ar_in = nc.dram_tensor("ar_in", [K, K + 2], F32)
    ar_out = nc.dram_tensor("ar_out", [C * K, K + 2], F32, addr_space="Shared")

    rg = [list(range(C))]
    AG = lambda i, o: nc.gpsimd.collective_compute(
        "AllGather", mybir.AluOpType.bypass, replica_groups=rg, ins=[i], outs=[o]
    )

    nc.gpsimd.load_library(library_config.mlp)

    with tile.TileContext(nc) as tc:
        with (
            tc.tile_pool(name="const", bufs=1) as cp,
            tc.tile_pool(name="tabs", bufs=1) as tp,
            tc.tile_pool(name="msg", bufs=3) as mp,
            tc.tile_pool(name="wt", bufs=10) as wtp,
            tc.tile_pool(name="work", bufs=2) as wk,
            tc.tile_pool(name="acc", bufs=1) as accp,
            tc.tile_pool(name="ps", bufs=2, space="PSUM") as ps,
            tc.tile_pool(name="psa", bufs=1, space="PSUM") as psa,
        ):
            # ---------------- constants / tables into SBUF
            def load(pool, name, src, shape, dtype=F32, eng=None):
                t = pool.tile(shape, dtype, tag=name)
                (eng or nc.sync).dma_start(out=t[:], in_=src)
                return t

            # deg_full first, on the ACT HWDGE queue: it gates dis -> xs ->
            # everything, while the SP queue drains the big edge tables.
            degf_sb = load(
                tp, "degftab", degf_t[:].rearrange("p (b l) -> p b l", l=LC),
                [P, NBLK, LC], BF16, eng=nc.scalar,
            )
            disf_sb = cp.tile([P, NBLK], F32, tag="disf")
            nc.vector.tensor_reduce(
                disf_sb[:], degf_sb[:], axis=mybir.AxisListType.X,
                op=mybir.AluOpType.add,
            )
            nc.scalar.sqrt(disf_sb[:], disf_sb[:])
            nc.vector.reciprocal(disf_sb[:], disf_sb[:])

            iota_sb = load(cp, "iota", iota_t[:], [P, P])
            iotab_sb = load(cp, "iotab", iotab_t[:], [P, P], BF16)
            ident_sb = load(cp, "ident", ident_t[:], [P, P])
            id64_sb = load(cp, "id64", id64_t[:], [K, K])
            ones_sb = load(cp, "ones", ones_t[:], [P, 1])
            ones_row_sb = load(cp, "ones_row", ones_row_t[:], [1, P])
            w1_sb = load(cp, "w1", w1[:], [P, FH])
            w2_sb = load(cp, "w2", w2[:].rearrange("(c p) f -> p c f", p=P), [P, 2, FH])
            wp_sb = load(cp, "wp", wp[:].rearrange("(c p) f -> p c f", p=P), [P, 2, K])
            b1_sb = load(cp, "b1", b1[:], [1, FH])
            b2_sb = load(cp, "b2", b2[:], [1, FH])
            bp_sb = load(cp, "bp", bp[:], [1, K])
            mask_sb = load(cp, "mask", mask_t[:], [P, BLK])
            gdloc_sb = load(tp, "gdloc", g_dloc[:], [P, BLK * TG])
            gw_sb = load(tp, "gw", g_w[:], [P, BLK * TG])
            gidx_sb = load(tp, "gidx", g_idx[:], [P, BLK * TG * 8], I16)
            gidx2_sb = load(tp, "gidx2", g_idx2[:], [P, BLK * TG * 8], I16)
            pdloc_sb = load(tp, "pdloc", p_dloc[:], [P, BLK * TP])
            pw_sb = load(tp, "pw", p_w[:], [P, BLK * TP])
            pidx_sb = load(tp, "pidx", p_idx[:], [P, BLK * TP * 8], I16)

            # ---------------- deg -> dis
            deg_sb = load(
                tp, "degtab", deg_t[:].rearrange("p (b l) -> p b l", l=LC),
                [P, BLK, LC], BF16,
            )
            dis_sb = cp.tile([P, BLK], F32, tag="dis")
            nc.vector.tensor_reduce(
                dis_sb[:], deg_sb[:], axis=mybir.AxisListType.X, op=mybir.AluOpType.add
            )
            nc.scalar.sqrt(dis_sb[:], dis_sb[:])
            nc.vector.reciprocal(dis_sb[:], dis_sb[:])

            rowdeg_sb = load(
                tp, "rowdegtab", rowdeg_t[:].rearrange("p (b l) -> p b l", l=LR),
                [P, BLK, LR],
            )
            d_sb = cp.tile([P, BLK], F32, tag="d")
            nc.vector.tensor_reduce(
                d_sb[:], rowdeg_sb[:], axis=mybir.AxisListType.X, op=mybir.AluOpType.add
            )

            # ---------------- x_scaled: full, local (x and deg_full replicated)
            XCH = 20  # blocks per x-scale chunk
            x_dr = x_fl[:].rearrange("(b p) f -> p b f", p=P)
            xs_dr = xs_full[:].rearrange("(b p) f -> p b f", p=P)
            for ch in range(NBLK // XCH):
                x_sb = mp.tile([P, XCH, FIN], BF16, tag="xin")
                nc.scalar.dma_start(
                    out=x_sb[:], in_=x_dr[:, ch * XCH : (ch + 1) * XCH, :]
                )
                xs_sb = mp.tile([P, XCH, FIN], BF16, tag="xs")
                for j in range(XCH):
                    B = ch * XCH + j
                    if j % 2 == 0:
                        nc.vector.tensor_scalar_mul(
                            xs_sb[:, j, :], x_sb[:, j, :], disf_sb[:, B : B + 1]
                        )
                    else:
                        nc.scalar.activation(
                            xs_sb[:, j, :], x_sb[:, j, :],
                            mybir.ActivationFunctionType.Copy,
                            scale=disf_sb[:, B : B + 1],
                        )
                nc.sync.dma_start(
                    out=xs_dr[:, ch * XCH : (ch + 1) * XCH, :], in_=xs_sb[:]
                )

            # ---------------- shared per-layer machinery
            def scatter_layer(src_dram, Fsrc, idx_sb, dloc_sb, w_sb, T, b, dt, io):
                """Gather block b's edge sources and scatter-accumulate into
                PSUM [128 dst, Fsrc] via one-hot matmuls. Returns psum tile."""
                msg = mp.tile([P, T, Fsrc], dt, tag="msg")
                nc.gpsimd.dma_gather(
                    msg[:],
                    src_dram,
                    idx_sb[:, b * T * 8 : (b + 1) * T * 8],
                    T * P,
                    T * P,
                    Fsrc,
                    single_packet=False,
                )
                psum = ps.tile([P, Fsrc], F32, tag="scat")
                for t in range(T):
                    wt = wtp.tile([P, P], dt, tag="onehot")
                    nc.vector.tensor_scalar(
                        wt[:],
                        io[:],
                        dloc_sb[:, b * T + t : b * T + t + 1],
                        w_sb[:, b * T + t : b * T + t + 1],
                        op0=mybir.AluOpType.is_equal,
                        op1=mybir.AluOpType.mult,
                    )
                    nc.tensor.matmul(
                        psum[:],
                        wt[:],
                        msg[:, t, :],
                        start=(t == 0),
                        stop=(t == T - 1),
                    )
                return psum

            def dense_after_scatter(psum_scat, Fsrc, wchunks_sb, Fout, bias_sb, b):
                """out_psum [128n, Fout] = (dis*psum_scat) @ W + bias."""
                sc = wk.tile([P, Fsrc], F32, tag="sc")
                nc.vector.tensor_scalar_mul(sc[:], psum_scat[:], dis_sb[:, b : b + 1])
                nch = Fsrc // P
                h_psum = ps.tile([P, Fout], F32, tag="mm")
                for c_ in range(nch):
                    tr = ps.tile([P, P], F32, tag="tr")
                    nc.tensor.transpose(
                        tr[:], sc[:, c_ * P : (c_ + 1) * P], ident_sb[:]
                    )
                    tr_sb = wk.tile([P, P], F32, tag="tr_sb")
                    nc.vector.tensor_copy(tr_sb[:], tr[:])
                    rhs = (
                        wchunks_sb[:, c_, :] if nch > 1 else wchunks_sb[:, :Fout]
                    )
                    nc.tensor.matmul(
                        h_psum[:], tr_sb[:], rhs, start=(c_ == 0), stop=False
                    )
                nc.tensor.matmul(
                    h_psum[:], ones_row_sb[:], bias_sb[:], start=False, stop=True
                )
                return h_psum

            # ---------------- layer 1
            y1_sb = wk.tile([P, BLK, FH], BF16, tag="y1")
            nc.vector.memset(y1_sb[:], 0.0)
            y1_dr = y1_in[:].rearrange("(b p) f -> p b f", p=P)
            if _MAX_PHASE >= 2:
                for b in range(BLK):
                    psc = scatter_layer(
                        xs_full[:], FIN, gidx_sb, gdloc_sb, gw_sb, TG, b,
                        BF16, iotab_sb,
                    )
                    h1 = dense_after_scatter(psc, FIN, w1_sb, FH, b1_sb, b)
                    nc.scalar.activation(
                        y1_sb[:, b, :],
                        h1[:],
                        mybir.ActivationFunctionType.Relu,
                        scale=dis_sb[:, b : b + 1],
                    )
                    if b == K1 - 1:
                        nc.sync.dma_start(
                            out=y1_dr[:, :K1, :], in_=y1_sb[:, :K1, :]
                        )
                        if _MAX_PHASE >= 3:
                            AG(y1_in[: K1 * P, :], y1_full[: C * K1 * P, :])
                nc.sync.dma_start(out=y1_dr[:, K1:, :], in_=y1_sb[:, K1:, :])
            if _MAX_PHASE >= 3:
                AG(y1_in[K1 * P :, :], y1_full[C * K1 * P :, :])
                if _DEBUG_OUTPUTS:
                    nc.sync.dma_start(out=dbg["y1"][:], in_=y1_full[:])

            # ---------------- layer 2 + softmax
            s_sb = accp.tile([P, BLK, K], F32, tag="s")
            ssq_sb = accp.tile([P, BLK], F32, tag="ssq")
            sscratch = wk.tile([P, K], F32, tag="sscratch")
            nc.vector.memset(s_sb[:], 0.0)
            nc.vector.memset(ssq_sb[:], 0.0)
            s_dr = s_in[:].rearrange("(b p) k -> p b k", p=P)
            for b in range(BLK if _MAX_PHASE >= 4 else 0):
                psc = scatter_layer(
                    y1_full[:], FH, gidx2_sb, gdloc_sb, gw_sb, TG, b, BF16, iotab_sb
                )
                h2 = dense_after_scatter(psc, FH, w2_sb, FH, b2_sb, b)
                o2 = wk.tile([P, FH], F32, tag="o2")
                nc.scalar.activation(
                    o2[:], h2[:], mybir.ActivationFunctionType.Relu
                )
                # s = softmax(o2 @ Wp + bp) * mask
                sp = ps.tile([P, K], F32, tag="mm")
                for c_ in range(2):
                    tr = ps.tile([P, P], F32, tag="tr")
                    nc.tensor.transpose(
                        tr[:], o2[:, c_ * P : (c_ + 1) * P], ident_sb[:]
                    )
                    tr_sb = wk.tile([P, P], F32, tag="tr_sb")
                    nc.vector.tensor_copy(tr_sb[:], tr[:])
                    nc.tensor.matmul(
                        sp[:], tr_sb[:], wp_sb[:, c_, :], start=(c_ == 0), stop=False
                    )
                nc.tensor.matmul(
                    sp[:], ones_row_sb[:], bp_sb[:], start=False, stop=True
                )
                smax = wk.tile([P, 1], F32, tag="smax")
                nc.vector.tensor_reduce(
                    smax[:], sp[:], axis=mybir.AxisListType.X, op=mybir.AluOpType.max,
                    negate=True,
                )
                sexp = wk.tile([P, K], F32, tag="sexp")
                ssum = wk.tile([P, 1], F32, tag="ssum")
                nc.scalar.activation(
                    sexp[:], sp[:], mybir.ActivationFunctionType.Exp,
                    bias=smax[:], accum_out=ssum[:],
                )
                nc.vector.reciprocal(ssum[:], ssum[:])
                nc.vector.tensor_scalar(
                    s_sb[:, b, :], sexp[:], ssum[:], mask_sb[:, b : b + 1],
                    op0=mybir.AluOpType.mult, op1=mybir.AluOpType.mult,
                )
                nc.scalar.activation(
                    sscratch[:], s_sb[:, b, :], mybir.ActivationFunctionType.Square,
                    accum_out=ssq_sb[:, b : b + 1],
                )
                if b == K2 - 1:
                    nc.sync.dma_start(out=s_dr[:, :K2, :], in_=s_sb[:, :K2, :])
                    if _MAX_PHASE >= 5:
                        AG(s_in[: K2 * P, :], s_full[: C * K2 * P, :])
            if _MAX_PHASE >= 4:
                nc.sync.dma_start(out=s_dr[:, K2:, :], in_=s_sb[:, K2:, :])
            if _MAX_PHASE >= 5:
                AG(s_in[K2 * P :, :], s_full[C * K2 * P :, :])
                if _DEBUG_OUTPUTS:
                    nc.sync.dma_start(out=dbg["s"][:], in_=s_full[:])

            # ---------------- pool phase: adj@s, num/den accumulators
            num_sb = accp.tile([P, BLK], F32, tag="num")
            nscratch = wk.tile([P, K], F32, tag="nscratch")
            nc.vector.memset(num_sb[:], 0.0)
            for b in range(BLK if _MAX_PHASE >= 6 else 0):
                pp = scatter_layer(
                    s_full[:], K, pidx_sb, pdloc_sb, pw_sb, TP, b, F32, iota_sb
                )
                nc.vector.tensor_tensor(
                    out=nscratch[:], in0=s_sb[:, b, :], in1=pp[:],
                    op=mybir.AluOpType.mult,
                )
                nc.vector.tensor_reduce(
                    num_sb[:, b : b + 1], nscratch[:],
                    axis=mybir.AxisListType.X, op=mybir.AluOpType.add,
                )

            if _MAX_PHASE >= 7:
                # ---------------- packed partial reduce: [ss | num | den]
                # ss partial from the LOCAL s shard (10 matmuls, no sfull DMA);
                # one AllGather (cheaper than AllReduce) + local sum of 8 chunks.
                ss_psum = psa.tile([K, K], F32, tag="ss")
                smalls = psa.tile([P, 8], F32, tag="smalls")
                for b in range(BLK):
                    nc.tensor.matmul(
                        ss_psum[:], s_sb[:, b, :], s_sb[:, b, :],
                        start=(b == 0), stop=(b == BLK - 1),
                    )
                red = wk.tile([P, 1], F32, tag="red")
                nc.vector.tensor_reduce(
                    red[:], num_sb[:], axis=mybir.AxisListType.X, op=mybir.AluOpType.add
                )
                num_ps = smalls[0:1, 0:1]
                nc.tensor.matmul(num_ps, red[:], ones_sb[:], start=True, stop=True)
                den_sb = wk.tile([P, BLK], F32, tag="den")
                nc.vector.tensor_tensor(
                    out=den_sb[:], in0=ssq_sb[:], in1=d_sb[:], op=mybir.AluOpType.mult
                )
                red2 = wk.tile([P, 1], F32, tag="red2")
                nc.vector.tensor_reduce(
                    red2[:], den_sb[:], axis=mybir.AxisListType.X, op=mybir.AluOpType.add
                )
                den_ps = smalls[0:1, 1:2]
                nc.tensor.matmul(den_ps, red2[:], ones_sb[:], start=True, stop=True)

                arbuf = wk.tile([K, K + 2], F32, tag="arbuf")
                nc.vector.memset(arbuf[:], 0.0)
                nc.vector.tensor_copy(arbuf[:, 0:K], ss_psum[:])
                nc.vector.tensor_copy(arbuf[0:1, K : K + 1], num_ps)
                nc.vector.tensor_copy(arbuf[0:1, K + 1 : K + 2], den_ps)
                nc.sync.dma_start(out=ar_in[:], in_=arbuf[:])
                AG(ar_in[:], ar_out[:])
                gath = wk.tile([K, C, K + 2], F32, tag="gath")
                nc.sync.dma_start(
                    out=gath[:], in_=ar_out[:].rearrange("(c r) f -> r c f", r=K)
                )
                acc = wk.tile([K, K + 2], F32, tag="acc")
                nc.vector.tensor_copy(acc[:], gath[:, 0, :])
                for c_ in range(1, C):
                    nc.vector.tensor_tensor(
                        out=acc[:], in0=acc[:], in1=gath[:, c_, :],
                        op=mybir.AluOpType.add,
                    )
                ss_sb = acc[:, 0:K]
                ndg_sb = acc[0:1, K : K + 2]
                if _DEBUG_OUTPUTS:
                    nc.sync.dma_start(out=dbg["ss"][:], in_=ss_sb)
                    nc.sync.dma_start(out=dbg["numden"][:], in_=ndg_sb)

                # ---------------- ortho loss + final scalar
                sq64 = wk.tile([K, K], F32, tag="sq64")
                col64 = wk.tile([K, 1], F32, tag="col64")
                nc.scalar.activation(
                    sq64[:], ss_sb, mybir.ActivationFunctionType.Square,
                    accum_out=col64[:],
                )
                fro_ps = smalls[0:1, 2:3]
                nc.tensor.matmul(fro_ps, col64[:], ones_sb[:K, :], start=True, stop=True)
                fro = wk.tile([1, 1], F32, tag="fro_sb")
                nc.scalar.sqrt(fro[:], fro_ps)
                nc.vector.reciprocal(fro[:], fro[:])
                # broadcast 1/fro to K partitions via rank-1 matmul
                fro_bc = smalls[0:K, 3:4]
                nc.tensor.matmul(
                    fro_bc, ones_row_sb[:, :K], fro[:], start=True, stop=True
                )
                fro64 = wk.tile([K, 1], F32, tag="fro64")
                nc.vector.tensor_copy(fro64[:], fro_bc)
                # t = ss/fro - I/sqrt(K)
                tmat = wk.tile([K, K], F32, tag="tmat")
                nc.vector.tensor_scalar_mul(tmat[:], ss_sb, fro64[:])
                nc.vector.tensor_tensor(
                    out=tmat[:], in0=tmat[:], in1=id64_sb[:],
                    op=mybir.AluOpType.subtract,
                )
                nc.scalar.activation(
                    sq64[:], tmat[:], mybir.ActivationFunctionType.Square,
                    accum_out=col64[:],
                )
                orth_ps = smalls[0:1, 4:5]
                nc.tensor.matmul(orth_ps, col64[:], ones_sb[:K, :], start=True, stop=True)
                orth = wk.tile([1, 1], F32, tag="orth_sb")
                nc.scalar.sqrt(orth[:], orth_ps)

                rden = wk.tile([1, 1], F32, tag="rden")
                nc.vector.reciprocal(rden[:], acc[0:1, K + 1 : K + 2])
                mloss = wk.tile([1, 1], F32, tag="mloss")
                nc.vector.tensor_tensor(
                    out=mloss[:], in0=acc[0:1, K : K + 1], in1=rden[:],
                    op=mybir.AluOpType.mult,
                )
                res = wk.tile([1, 1], F32, tag="res")
                nc.vector.tensor_tensor(
                    out=res[:], in0=orth[:], in1=mloss[:], op=mybir.AluOpType.subtract
                )
                nc.sync.dma_start(out=out_t[:], in_=res[:])
            else:
                nc.sync.dma_start(out=out_t[:], in_=dis_sb[0:1, 0:1])

    if not for_sim:
        _split_excess_waits(nc)
    lower_extended_insts(nc)
    return nc


_PROG_CACHE = {}


def _get_program(key):
    if key not in _PROG_CACHE:
        _PROG_CACHE[key] = build_program(*key)
    return _PROG_CACHE[key]


def make_in_maps(inputs, tabs):
    x = np.asarray(inputs["x"], np.float32)
    W1, W2, Wp = inputs["W1"], inputs["W2"], inputs["Wp"]
    b1, b2, bp = inputs["b1"], inputs["b2"], inputs["bp"]
    xpad = np.zeros((NPAD, FIN), np.float32)
    xpad[:N] = x
    iota = np.tile(np.arange(P, dtype=np.float32), (P, 1))
    ident = np.eye(P, dtype=np.float32)
    id64e = (np.eye(K, dtype=np.float32) / np.sqrt(np.float32(K))).astype(np.float32)
    ones = np.ones((P, 1), np.float32)

    common = dict(
        W1=np.asarray(W1, np.float32),
        W2=np.asarray(W2, np.float32),
        Wp=np.asarray(Wp, np.float32),
        b1=np.asarray(b1, np.float32).reshape(1, FH),
        b2=np.asarray(b2, np.float32).reshape(1, FH),
        bp=np.asarray(bp, np.float32).reshape(1, K),
        iota=iota,
        iotab=iota.astype(NPBF16),
        ident=ident,
        id64e=id64e,
        ones=ones,
        ones_row=np.ones((1, P), np.float32),
    )
    in_maps = []
    for c in range(C):
        in_maps.append(
            dict(
                common,
                x_full=xpad.astype(NPBF16),
                deg_full=tabs["deg_full"],
                g_idx=tabs["g_idx"][c],
                g_idx2=tabs["g_idx2"][c],
                g_dloc=tabs["g_dloc"][c],
                g_w=tabs["g_w"][c],
                p_idx=tabs["p_idx"][c],
                p_dloc=tabs["p_dloc"][c],
                p_w=tabs["p_w"][c],
                deg=tabs["deg"][c],
                rowdeg=tabs["rowdeg"][c],
                mask=tabs["mask"][c],
            )
        )
    return in_maps


def kernel(x, edge_index, edge_weight, W1, b1, W2, b2, Wp, bp):
    edge_index = np.asarray(edge_index)
    edge_weight = np.asarray(edge_weight, np.float32)
    tabs = preprocess(edge_index, edge_weight)
    nc = _get_program((tabs["TG"], tabs["TP"], tabs["LC"], tabs["LR"]))
    in_maps = make_in_maps(
        dict(x=x, W1=W1, b1=b1, W2=W2, b2=b2, Wp=Wp, bp=bp), tabs
    )
    trace = bool(int(os.environ.get("KERNEL_TRACE", "0")))
    kwargs = {}
    if trace:
        kwargs = dict(trace=True, tmpdir=os.environ.get("KERNEL_TRACE_DIR"))
    res = run_bass_kernel_spmd(nc, in_maps, core_ids=list(range(C)), **kwargs)
    if trace:
        kernel.exec_time_ns = res.exec_time_ns
        kernel.mean_exec_time_ns = res.mean_exec_time_ns
        kernel.bass_results = res
    out = res.results[0]["out"].reshape(())
    if _DEBUG_OUTPUTS:
        kernel.debug = {k: res.results[0][f"dbg_{k}"] for k in ("y1", "s", "numden", "ss")}
    return np.float32(out)


if __name__ == "__main__":
    import reference

    inputs = reference.setup_inputs()
    inputs = {k: np.asarray(v) for k, v in inputs.items()}
    got = kernel(**inputs)
    print("kernel out:", got)



# revision 22
# speedup vs baseline: 2.8118x; 2.8118x over previous
"""MinCutNet (2x GCN + dense_mincut_pool losses) as an 8-core Trainium2
Bass/Tile kernel.

v2: dense-operator design. Host builds the GCN-normalized operator
M[src, dst] = dis[src] * (A + I)[src, dst] * dis[dst] once (bf16), sharded
column-wise (dst) across the 8 cores: per core a [128, 80*10*128] slab table.
All three aggregations (GCN layer 1, GCN layer 2, mincut pool numerator) are
blocked dense matmuls against streamed M tiles — no SWDGE gathers, no
per-edge descriptor generation. The pool numerator uses
  tr(S^T adj S) = sum(shat * aggM(shat)) - sum(s*s),  shat = sqrt(deg) * s,
which reuses the SAME M table (adj + I = D^1/2 M D^1/2).

Cross-core: AllGather of the layer-1 aggregate (feature-major, 2.6MB),
AllGather of shat (1.3MB bf16), and the tiny packed [ss|num|den] AllGather
for the final scalar reduction.
"""

import os
import sys

sys.path.insert(0, "/opt/trn_rl_repo")

import numpy as np

import concourse.bass as bass
import concourse.mybir as mybir
import concourse.tile as tile
from concourse.bass_utils import run_bass_kernel_spmd
from concourse.library_overlay import lower_extended_insts
from concourse.vector_clock import ScopedClock

# ---------------------------------------------------------------- constants
N, E = 10000, 320000
FIN, FH, K = 128, 256, 64
C = 8               # cores
P = 128             # partitions
NPAD = 10240        # 80 blocks of 128
SHARD = NPAD // C   # 1280 nodes per core
BLK = SHARD // P    # 10 dst blocks per core
NBLK = NPAD // P    # 80 src blocks total
YCH = 256           # node-chunk width for the replicated y1 dense layer
F32 = mybir.dt.float32
BF16 = mybir.dt.bfloat16
import ml_dtypes

NPBF16 = ml_dtypes.bfloat16

_DEBUG_OUTPUTS = bool(int(os.environ.get("KERNEL_DEBUG_OUTPUTS", "0")))


# ------------------------------------------------------- tile drain patch
def _patched_drain_and_barrier(self, tick_clock, wait_clock):
    """walrus in this container rejects >1 sync-wait command on the tail
    Drain; spread the waits across SP nops (1 wait each)."""
    nc = self.nc
    drain_inst = nc.sync.drain()
    wait_clock.add_sem_waits(
        drain_inst.ins, ScopedClock({None: tick_clock.global_clock})
    )
    waits = list(drain_inst.ins.sync_info.on_wait)
    if len(waits) > 1:
        upd = list(drain_inst.ins.sync_info.on_update)
        drain_inst.ins.sync_info = mybir.SyncInfo(on_wait=waits[:1], on_update=upd)
        for i, w in enumerate(waits[1:]):
            nop = nc.sync.nop(nofuse=True, hint=f"tailwait{i}")
            nop.ins.sync_info = mybir.SyncInfo(on_wait=[w], on_update=[])
    nc.all_engine_barrier()
    assert self.sems is not None
    popped = nc._tile_sem_poison_stack.pop()
    assert popped is self._sem_poison
    nc.clear_and_free_semaphores(list(self.sems.allocated().values()))
    nc.all_engine_barrier()


tile.TileContext._drain_and_barrier = _patched_drain_and_barrier

_noop_ctr = [0]


def _split_excess_waits(nc, lim=1):
    """walrus in this container caps sync-wait commands per instruction;
    spill excess waits onto same-engine NOPs placed just before."""
    nsplit = 0
    for fn in nc.m.functions:
        for b in fn.blocks:
            newl = []
            changed = False
            for inst in b.instructions:
                si = inst.sync_info
                if si is not None and len(si.on_wait) > lim:
                    waits = list(si.on_wait)
                    head, tail = waits[: len(waits) - lim], waits[len(waits) - lim :]
                    for i in range(0, len(head), lim):
                        _noop_ctr[0] += 1
                        nop = mybir.InstNoOp(
                            name=f"waitnop-{_noop_ctr[0]}",
                            sync_info=mybir.SyncInfo(
                                on_wait=head[i : i + lim], on_update=[]
                            ),
                            bass_nofuse=True,
                            engine=inst.engine,
                        )
                        newl.append(nop)
                    inst.sync_info = mybir.SyncInfo(
                        on_wait=tail, on_update=list(si.on_update)
                    )
                    nsplit += 1
                    changed = True
                newl.append(inst)
            if changed:
                b.instructions = newl
    return nsplit


# ------------------------------------------------------- host preprocessing
def preprocess(edge_index, edge_weight):
    row = edge_index[0].astype(np.int64)
    col = edge_index[1].astype(np.int64)
    ew = edge_weight.astype(np.float32)

    deg = np.zeros(NPAD, np.float32)
    np.add.at(deg, col, ew)
    deg[:N] += 1.0  # self loops
    dis = np.zeros(NPAD, np.float32)
    nz = deg > 0
    dis[nz] = 1.0 / np.sqrt(deg[nz])
    sqdeg = np.sqrt(deg)

    # dense normalized operator M[s, d] = dis[s] * (A + I)[s, d] * dis[d]
    M = np.zeros((NPAD, NPAD), np.float32)
    np.add.at(M, (row, col), ew)
    idx = np.arange(N)
    M[idx, idx] += 1.0
    M *= dis[:, None]
    M *= dis[None, :]

    # per-core slab tables: mt[c][p, (bp*BLK + b)*P + q] = M[bp*P+p, c*SHARD+b*P+q]
    Mr = M.reshape(NBLK, P, C, BLK * P)
    mt = np.empty((C, P, NBLK * BLK * P), NPBF16)
    for c in range(C):
        mt[c] = (
            np.ascontiguousarray(Mr[:, :, c, :].transpose(1, 0, 2))
            .reshape(P, NBLK * BLK * P)
            .astype(NPBF16)
        )

    d_row = np.zeros(NPAD, np.float32)
    np.add.at(d_row, row, ew)

    mask = np.zeros(NPAD, np.float32)
    mask[:N] = 1.0

    def shard_cols(v):
        # [NPAD] -> [C, P, BLK]  ([p, b] = v[c*SHARD + b*P + p])
        return np.ascontiguousarray(v.reshape(C, BLK, P).transpose(0, 2, 1))

    return dict(
        mt=mt,
        d_row=shard_cols(d_row),
        sqdeg=shard_cols(sqdeg),
        mask=shard_cols(mask),
    )


# --------------------------------------------------------- device program
def build_program(for_sim=False):
    nc = bass.Bass(num_devices=C)
    dp = nc.declare_dram_parameter

    x_fl = dp("x_full", [NPAD, FIN], BF16, isOutput=False)
    mt = dp("mt", [P, NBLK * BLK * P], BF16, isOutput=False)
    w1 = dp("w1t", [P, 2 * P], BF16, isOutput=False)      # [fi, fo_c, fo]
    w2 = dp("w2t", [P, 2 * FH], BF16, isOutput=False)     # [fi_p, fi_c, fo]
    wp = dp("wpt", [P, 2 * K], F32, isOutput=False)       # [fo_p, fo_c, k]
    b1t = dp("b1t", [P, 2], F32, isOutput=False)
    b2r = dp("b2r", [1, FH], F32, isOutput=False)
    bpr = dp("bpr", [1, K], F32, isOutput=False)
    drow_t = dp("drow", [P, BLK], F32, isOutput=False)
    sqdeg_t = dp("sqdeg", [P, BLK], F32, isOutput=False)
    mask_t = dp("mask", [P, BLK], F32, isOutput=False)
    ident_t = dp("ident", [P, P], F32, isOutput=False)
    id64_t = dp("id64e", [K, K], F32, isOutput=False)  # I/sqrt(K)
    ones_t = dp("ones", [P, 1], F32, isOutput=False)
    ones_row_t = dp("ones_row", [1, P], F32, isOutput=False)

    out_t = dp("out", [1, 1], F32, isOutput=True)
    dbg = {}
    if _DEBUG_OUTPUTS:
        dbg["ag1"] = dp("dbg_ag1", [P, C * SHARD], BF16, isOutput=True)
        dbg["sh"] = dp("dbg_sh", [NPAD, K], BF16, isOutput=True)
        dbg["numden"] = dp("dbg_numden", [1, 2], F32, isOutput=True)
        dbg["ss"] = dp("dbg_ss", [K, K], F32, isOutput=True)

    # internal DRAM
    ag1_in = nc.dram_tensor("ag1_in", [P, SHARD], BF16)
    ag1_out = nc.dram_tensor("ag1_out", [C * P, SHARD], BF16, addr_space="Shared")
    sh_in = nc.dram_tensor("sh_in", [SHARD, K], BF16)
    sh_full = nc.dram_tensor("sh_full", [NPAD, K], BF16, addr_space="Shared")
    ar_in = nc.dram_tensor("ar_in", [K, K + 2], F32)
    ar_out = nc.dram_tensor("ar_out", [C * K, K + 2], F32, addr_space="Shared")

    rg = [list(range(C))]
    AG = lambda i, o: nc.gpsimd.collective_compute(
        "AllGather", mybir.AluOpType.bypass, replica_groups=rg, ins=[i], outs=[o]
    )

    with tile.TileContext(nc) as tc:
        with (
            tc.tile_pool(name="const", bufs=1) as cp,
            tc.tile_pool(name="big", bufs=1) as bigp,
            tc.tile_pool(name="mslab", bufs=3) as mp,
            tc.tile_pool(name="work", bufs=2) as wk,
            tc.tile_pool(name="acc", bufs=1) as accp,
            tc.tile_pool(name="ps", bufs=5, space="PSUM") as ps,
            tc.tile_pool(name="pss", bufs=1, space="PSUM") as pss,
            tc.tile_pool(name="psa", bufs=1, space="PSUM") as psa,
        ):
            def load(pool, name, src, shape, dtype=F32, eng=None):
                t = pool.tile(shape, dtype, tag=name)
                (eng or nc.sync).dma_start(out=t[:], in_=src)
                return t

            # x first on the ACT queue: it gates layer 1 while SP streams slabs.
            x_sb = load(
                bigp, "xsb", x_fl[:].rearrange("(b p) f -> p b f", p=P),
                [P, NBLK, FIN], BF16, eng=nc.scalar,
            )
            w1_sb = load(cp, "w1", w1[:].rearrange("p (c f) -> p c f", c=2),
                         [P, 2, P], BF16, eng=nc.scalar)
            w2_sb = load(cp, "w2", w2[:].rearrange("p (c f) -> p c f", c=2),
                         [P, 2, FH], BF16, eng=nc.scalar)
            wp_sb = load(cp, "wp", wp[:].rearrange("p (c f) -> p c f", c=2),
                         [P, 2, K], F32, eng=nc.scalar)
            b1_sb = load(cp, "b1t", b1t[:], [P, 2], F32, eng=nc.scalar)
            b2_sb = load(cp, "b2r", b2r[:], [1, FH], F32, eng=nc.scalar)
            bp_sb = load(cp, "bpr", bpr[:], [1, K], F32, eng=nc.scalar)
            drow_sb = load(cp, "drow", drow_t[:], [P, BLK], F32, eng=nc.scalar)
            sqdeg_sb = load(cp, "sqdeg", sqdeg_t[:], [P, BLK], F32, eng=nc.scalar)
            mask_sb = load(cp, "mask", mask_t[:], [P, BLK], F32, eng=nc.scalar)
            ident_sb = load(cp, "ident", ident_t[:], [P, P], F32, eng=nc.scalar)
            id64_sb = load(cp, "id64", id64_t[:], [K, K], F32, eng=nc.scalar)
            ones_sb = load(cp, "ones", ones_t[:], [P, 1], F32, eng=nc.scalar)
            ones_row_sb = load(cp, "ones_row", ones_row_t[:], [1, P], F32,
                               eng=nc.scalar)

            mt_dr = mt[:].rearrange("p (s b q) -> p s (b q)", s=NBLK, q=P)

            import itertools
            _slabctr = itertools.count()

            def slab_load(bp_):
                slab = mp.tile([P, BLK, P], BF16, tag="slab", name=f"slab{next(_slabctr)}")
                nc.sync.dma_start(
                    out=slab[:],
                    in_=mt_dr[:, bp_, :].rearrange("p (b q) -> p b q", q=P),
                )
                return slab

            # ---------------- layer 1 aggregation: agg1T[fi, dst] = (M^T X)^T
            agg1t = [
                ps.tile([P, 4, P], F32, tag="acc2k", name=f"agg1_{i}")
                for i in range(3)
            ]
            agg1 = [agg1t[b // 4][:, b % 4, :] for b in range(BLK)]
            for bp_ in range(NBLK):
                slab = slab_load(bp_)
                for b in range(BLK):
                    # start=True clears has_written for the WHOLE bank, so only
                    # the first matmul touching each packed psum bank may set it;
                    # other sub-regions rely on overwrite-where-unset.
                    nc.tensor.matmul(
                        agg1[b], x_sb[:, bp_, :], slab[:, b, :],
                        start=(bp_ == 0 and b % 4 == 0),
                        stop=(bp_ == NBLK - 1),
                    )
            a1sb = wk.tile([P, BLK, P], BF16, tag="a1sb")
            for b in range(BLK):
                nc.vector.tensor_copy(a1sb[:, b, :], agg1[b])
            nc.sync.dma_start(
                out=ag1_in[:].rearrange("p (b q) -> p b q", q=P), in_=a1sb[:]
            )
            AG(ag1_in[:], ag1_out[:])

            # readback feature-major full aggregate [fi, node]
            a1T = bigp.tile([P, C * SHARD], BF16, tag="a1T")
            # AG-dependent readbacks go on the ACT queue so the SP queue's
            # slab streaming is not head-of-line blocked behind the AG wait.
            nc.scalar.dma_start(
                out=a1T[:].rearrange("f (c n) -> f c n", c=C),
                in_=ag1_out[:].rearrange("(c f) n -> f c n", f=P),
            )
            if _DEBUG_OUTPUTS:
                nc.scalar.dma_start(out=dbg["ag1"][:], in_=a1T[:])

            # ---------------- y1T = relu(W1^T agg1T + b1T)   [fo, node], bf16
            y1T = bigp.tile([P, 2, NPAD], BF16, tag="y1T")
            for fo_c in range(2):
                for ch in range(NPAD // YCH):
                    py = ps.tile([P, YCH], F32, tag="acc2k", name=f"py_{fo_c}_{ch}")
                    nc.tensor.matmul(
                        py[:], w1_sb[:, fo_c, :],
                        a1T[:, ch * YCH : (ch + 1) * YCH],
                        start=True, stop=True,
                    )
                    nc.scalar.activation(
                        y1T[:, fo_c, ch * YCH : (ch + 1) * YCH], py[:],
                        mybir.ActivationFunctionType.Relu,
                        bias=b1_sb[:, fo_c : fo_c + 1],
                    )

            # ---------------- z = y1 @ W2   [src, fo], bf16 (node-major)
            z_sb = bigp.tile([P, NBLK, FH], BF16, tag="zsb")
            for bp_ in range(NBLK):
                pz = ps.tile([P, FH], F32, tag="acc2k", name=f"pz_{bp_}")
                for fi_c in range(2):
                    nc.tensor.matmul(
                        pz[:], y1T[:, fi_c, bp_ * P : (bp_ + 1) * P],
                        w2_sb[:, fi_c, :],
                        start=(fi_c == 0), stop=(fi_c == 1),
                    )
                nc.vector.tensor_copy(z_sb[:, bp_, :], pz[:])

            # ---------------- layer 2 aggregation: h2[dst, fo] (node-major)
            h2t = [
                ps.tile([P, 2, FH], F32, tag="acc2k", name=f"h2_{i}")
                for i in range(5)
            ]
            h2 = [h2t[b // 2][:, b % 2, :] for b in range(BLK)]
            for bp_ in range(NBLK):
                slab = slab_load(bp_)
                for b in range(BLK):
                    nc.tensor.matmul(
                        h2[b], slab[:, b, :], z_sb[:, bp_, :],
                        start=(bp_ == 0 and b % 2 == 0), stop=False,
                    )
            for b in range(BLK):
                nc.tensor.matmul(
                    h2[b], ones_row_sb[:], b2_sb[:], start=False, stop=True
                )

            # ---------------- s = softmax(relu(h2) @ Wp + bp) per block
            s_sb = accp.tile([P, BLK, K], F32, tag="s")
            sh_sb = accp.tile([P, BLK, K], F32, tag="sh")
            shb_sb = accp.tile([P, BLK, K], BF16, tag="shb")
            ssq_sb = accp.tile([P, BLK], F32, tag="ssq")
            sscratch = wk.tile([P, K], F32, tag="sscratch")
            for b in range(BLK):
                o2 = wk.tile([P, FH], F32, tag="o2")
                nc.scalar.activation(
                    o2[:], h2[b], mybir.ActivationFunctionType.Relu
                )
                sp = pss.tile([P, K], F32, tag="sp")
                for c_ in range(2):
                    tr = pss.tile([P, P], F32, tag="tr")
                    nc.tensor.transpose(
                        tr[:], o2[:, c_ * P : (c_ + 1) * P], ident_sb[:]
                    )
                    tr_sb = wk.tile([P, P], F32, tag="tr_sb")
                    nc.vector.tensor_copy(tr_sb[:], tr[:])
                    nc.tensor.matmul(
                        sp[:], tr_sb[:], wp_sb[:, c_, :], start=(c_ == 0),
                        stop=False,
                    )
                nc.tensor.matmul(
                    sp[:], ones_row_sb[:], bp_sb[:], start=False, stop=True
                )
                smax = wk.tile([P, 1], F32, tag="smax")
                nc.vector.tensor_reduce(
                    smax[:], sp[:], axis=mybir.AxisListType.X,
                    op=mybir.AluOpType.max, negate=True,
                )
                sexp = wk.tile([P, K], F32, tag="sexp")
                ssum = wk.tile([P, 1], F32, tag="ssum")
                nc.scalar.activation(
                    sexp[:], sp[:], mybir.ActivationFunctionType.Exp,
                    bias=smax[:], accum_out=ssum[:],
                )
                nc.vector.reciprocal(ssum[:], ssum[:])
                nc.vector.tensor_scalar(
                    s_sb[:, b, :], sexp[:], ssum[:], mask_sb[:, b : b + 1],
                    op0=mybir.AluOpType.mult, op1=mybir.AluOpType.mult,
                )
                nc.scalar.activation(
                    sscratch[:], s_sb[:, b, :],
                    mybir.ActivationFunctionType.Square,
                    accum_out=ssq_sb[:, b : b + 1],
                )
                nc.vector.tensor_scalar_mul(
                    sh_sb[:, b, :], s_sb[:, b, :], sqdeg_sb[:, b : b + 1]
                )
                nc.vector.tensor_copy(shb_sb[:, b, :], sh_sb[:, b, :])
            nc.sync.dma_start(
                out=sh_in[:].rearrange("(b p) k -> p b k", p=P), in_=shb_sb[:]
            )
            AG(sh_in[:], sh_full[:])
            if _DEBUG_OUTPUTS:
                nc.scalar.dma_start(out=dbg["sh"][:], in_=sh_full[:])

            # ss = S^T S partial (local shard) — overlaps the AllGather
            psbig = psa.tile([P, K + 8], F32, tag="psbig")
            ss_psum = psbig[0:K, 0:K]
            smalls = psbig[:, K : K + 8]
            for b in range(BLK):
                nc.tensor.matmul(
                    ss_psum, s_sb[:, b, :], s_sb[:, b, :],
                    start=(b == 0), stop=(b == BLK - 1),
                )

            # ---------------- pool aggregation: hp[dst, k] = aggM(shat)
            shf = bigp.tile([P, NBLK, K], BF16, tag="shf")
            nc.scalar.dma_start(
                out=shf[:], in_=sh_full[:].rearrange("(b p) k -> p b k", p=P)
            )
            hpt = [
                ps.tile([P, 5, K], F32, tag="acc2k", name=f"hp_{i}")
                for i in range(2)
            ]
            hp = [hpt[b // 5][:, b % 5, :] for b in range(BLK)]
            for bp_ in range(NBLK):
                slab = slab_load(bp_)
                for b in range(BLK):
                    nc.tensor.matmul(
                        hp[b], slab[:, b, :], shf[:, bp_, :],
                        start=(bp_ == 0 and b % 5 == 0),
                        stop=(bp_ == NBLK - 1),
                    )
            num_sb = accp.tile([P, BLK], F32, tag="num")
            nscratch = wk.tile([P, K], F32, tag="nscratch")
            for b in range(BLK):
                nc.vector.tensor_tensor(
                    out=nscratch[:], in0=sh_sb[:, b, :], in1=hp[b],
                    op=mybir.AluOpType.mult,
                )
                nc.vector.tensor_reduce(
                    num_sb[:, b : b + 1], nscratch[:],
                    axis=mybir.AxisListType.X, op=mybir.AluOpType.add,
                )

            # ---------------- packed partial reduce: [ss | num | den]
            red = wk.tile([P, 1], F32, tag="red")
            nc.vector.tensor_reduce(
                red[:], num_sb[:], axis=mybir.AxisListType.X,
                op=mybir.AluOpType.add,
            )
            redq = wk.tile([P, 1], F32, tag="redq")
            nc.vector.tensor_reduce(
                redq[:], ssq_sb[:], axis=mybir.AxisListType.X,
                op=mybir.AluOpType.add,
            )
            # num_partial = sum(shat*aggM(shat)) - sum(s*s)
            nc.vector.tensor_tensor(
                out=red[:], in0=red[:], in1=redq[:], op=mybir.AluOpType.subtract
            )
            num_ps = smalls[0:1, 0:1]
            nc.tensor.matmul(num_ps, red[:], ones_sb[:], start=True, stop=True)
            den_sb = wk.tile([P, BLK], F32, tag="den")
            nc.vector.tensor_tensor(
                out=den_sb[:], in0=ssq_sb[:], in1=drow_sb[:],
                op=mybir.AluOpType.mult,
            )
            red2 = wk.tile([P, 1], F32, tag="red2")
            nc.vector.tensor_reduce(
                red2[:], den_sb[:], axis=mybir.AxisListType.X,
                op=mybir.AluOpType.add,
            )
            den_ps = smalls[0:1, 1:2]
            nc.tensor.matmul(den_ps, red2[:], ones_sb[:], start=True, stop=True)

            arbuf = wk.tile([K, K + 2], F32, tag="arbuf")
            nc.vector.memset(arbuf[:], 0.0)
            nc.vector.tensor_copy(arbuf[:, 0:K], ss_psum)
            nc.vector.tensor_copy(arbuf[0:1, K : K + 1], num_ps)
            nc.vector.tensor_copy(arbuf[0:1, K + 1 : K + 2], den_ps)
            nc.sync.dma_start(out=ar_in[:], in_=arbuf[:])
            AG(ar_in[:], ar_out[:])
            gath = wk.tile([K, C, K + 2], F32, tag="gath")
            nc.scalar.dma_start(
                out=gath[:], in_=ar_out[:].rearrange("(c r) f -> r c f", r=K)
            )
            acc = wk.tile([K, K + 2], F32, tag="acc")
            nc.vector.tensor_copy(acc[:], gath[:, 0, :])
            for c_ in range(1, C):
                nc.vector.tensor_tensor(
                    out=acc[:], in0=acc[:], in1=gath[:, c_, :],
                    op=mybir.AluOpType.add,
                )
            ss_sb = acc[:, 0:K]
            ndg_sb = acc[0:1, K : K + 2]
            if _DEBUG_OUTPUTS:
                nc.sync.dma_start(out=dbg["ss"][:], in_=ss_sb)
                nc.sync.dma_start(out=dbg["numden"][:], in_=ndg_sb)

            # ---------------- ortho loss + final scalar
            sq64 = wk.tile([K, K], F32, tag="sq64")
            col64 = wk.tile([K, 1], F32, tag="col64")
            nc.scalar.activation(
                sq64[:], ss_sb, mybir.ActivationFunctionType.Square,
                accum_out=col64[:],
            )
            fro_ps = smalls[0:1, 2:3]
            nc.tensor.matmul(fro_ps, col64[:], ones_sb[:K, :], start=True, stop=True)
            fro = wk.tile([1, 1], F32, tag="fro_sb")
            nc.scalar.sqrt(fro[:], fro_ps)
            nc.vector.reciprocal(fro[:], fro[:])
            # broadcast 1/fro to K partitions via rank-1 matmul
            fro_bc = smalls[0:K, 3:4]
            nc.tensor.matmul(
                fro_bc, ones_row_sb[:, :K], fro[:], start=True, stop=True
            )
            fro64 = wk.tile([K, 1], F32, tag="fro64")
            nc.vector.tensor_copy(fro64[:], fro_bc)
            # t = ss/fro - I/sqrt(K)
            tmat = wk.tile([K, K], F32, tag="tmat")
            nc.vector.tensor_scalar_mul(tmat[:], ss_sb, fro64[:])
            nc.vector.tensor_tensor(
                out=tmat[:], in0=tmat[:], in1=id64_sb[:],
                op=mybir.AluOpType.subtract,
            )
            nc.scalar.activation(
                sq64[:], tmat[:], mybir.ActivationFunctionType.Square,
                accum_out=col64[:],
            )
            orth_ps = smalls[0:1, 4:5]
            nc.tensor.matmul(orth_ps, col64[:], ones_sb[:K, :], start=True, stop=True)
            orth = wk.tile([1, 1], F32, tag="orth_sb")
            nc.scalar.sqrt(orth[:], orth_ps)

            rden = wk.tile([1, 1], F32, tag="rden")
            nc.vector.reciprocal(rden[:], acc[0:1, K + 1 : K + 2])
            mloss = wk.tile([1, 1], F32, tag="mloss")
            nc.vector.tensor_tensor(
                out=mloss[:], in0=acc[0:1, K : K + 1], in1=rden[:],
                op=mybir.AluOpType.mult,
            )
            res = wk.tile([1, 1], F32, tag="res")
            nc.vector.tensor_tensor(
                out=res[:], in0=orth[:], in1=mloss[:], op=mybir.AluOpType.subtract
            )
            nc.sync.dma_start(out=out_t[:], in_=res[:])

    if not for_sim:
        _split_excess_waits(nc)
    lower_extended_insts(nc)
    return nc


_PROG_CACHE = {}


def _get_program(key=()):
    if key not in _PROG_CACHE:
        _PROG_CACHE[key] = build_program()
    return _PROG_CACHE[key]


def make_in_maps(inputs, tabs):
    x = np.asarray(inputs["x"], np.float32)
    W1, W2, Wp = (np.asarray(inputs[k], np.float32) for k in ("W1", "W2", "Wp"))
    b1, b2, bp = (np.asarray(inputs[k], np.float32) for k in ("b1", "b2", "bp"))
    xpad = np.zeros((NPAD, FIN), np.float32)
    xpad[:N] = x
    ident = np.eye(P, dtype=np.float32)
    id64e = (np.eye(K, dtype=np.float32) / np.sqrt(np.float32(K))).astype(np.float32)

    common = dict(
        x_full=xpad.astype(NPBF16),
        w1t=W1.reshape(P, 2 * P).astype(NPBF16),
        w2t=W2.reshape(2, P, FH).transpose(1, 0, 2).reshape(P, 2 * FH).astype(NPBF16),
        wpt=Wp.reshape(2, P, K).transpose(1, 0, 2).reshape(P, 2 * K).astype(np.float32),
        b1t=np.ascontiguousarray(b1.reshape(2, P).T),
        b2r=b2.reshape(1, FH),
        bpr=bp.reshape(1, K),
        ident=ident,
        id64e=id64e,
        ones=np.ones((P, 1), np.float32),
        ones_row=np.ones((1, P), np.float32),
    )
    in_maps = []
    for c in range(C):
        in_maps.append(
            dict(
                common,
                mt=tabs["mt"][c],
                drow=tabs["d_row"][c],
                sqdeg=tabs["sqdeg"][c],
                mask=tabs["mask"][c],
            )
        )
    return in_maps


def kernel(x, edge_index, edge_weight, W1, b1, W2, b2, Wp, bp):
    edge_index = np.asarray(edge_index)
    edge_weight = np.asarray(edge_weight, np.float32)
    tabs = preprocess(edge_index, edge_weight)
    nc = _get_program()
    in_maps = make_in_maps(
        dict(x=x, W1=W1, b1=b1, W2=W2, b2=b2, Wp=Wp, bp=bp), tabs
    )
    trace = bool(int(os.environ.get("KERNEL_TRACE", "0")))
    kwargs = {}
    if trace:
        kwargs = dict(trace=True, tmpdir=os.environ.get("KERNEL_TRACE_DIR"))
    res = run_bass_kernel_spmd(nc, in_maps, core_ids=list(range(C)), **kwargs)
    if trace:
        kernel.exec_time_ns = res.exec_time_ns
        kernel.mean_exec_time_ns = res.mean_exec_time_ns
        kernel.bass_results = res
    out = res.results[0]["out"].reshape(())
    if _DEBUG_OUTPUTS:
        kernel.debug = {
            k: res.results[0][f"dbg_{k}"] for k in ("ag1", "sh", "numden", "ss")
        }
    return np.float32(out)


if __name__ == "__main__":
    import reference

    inputs = reference.setup_inputs()
    inputs = {k: np.asarray(v) for k, v in inputs.items()}
    got = kernel(**inputs)
    print("kernel out:", got)


# revision 31
# speedup vs baseline: 3.1960x; 1.1367x over previous
"""MinCutNet (2x GCN + dense_mincut_pool losses) as an 8-core Trainium2
Bass/Tile kernel.

v2: dense-operator design. Host builds the GCN-normalized operator
M[src, dst] = dis[src] * (A + I)[src, dst] * dis[dst] once (bf16), sharded
column-wise (dst) across the 8 cores: per core a [128, 80*10*128] slab table.
All three aggregations (GCN layer 1, GCN layer 2, mincut pool numerator) are
blocked dense matmuls against streamed M tiles — no SWDGE gathers, no
per-edge descriptor generation. The pool numerator uses
  tr(S^T adj S) = sum(shat * aggM(shat)) - sum(s*s),  shat = sqrt(deg) * s,
which reuses the SAME M table (adj + I = D^1/2 M D^1/2).

Cross-core: AllGather of the layer-1 aggregate (feature-major, 2.6MB),
AllGather of shat (1.3MB bf16), and the tiny packed [ss|num|den] AllGather
for the final scalar reduction.
"""

import os
import sys

sys.path.insert(0, "/opt/trn_rl_repo")

import numpy as np

import concourse.bass as bass
import concourse.mybir as mybir
import concourse.tile as tile
from concourse.bass_utils import run_bass_kernel_spmd
from concourse.library_overlay import lower_extended_insts
from concourse.vector_clock import ScopedClock

# ---------------------------------------------------------------- constants
N, E = 10000, 320000
FIN, FH, K = 128, 256, 64
C = 8               # cores
P = 128             # partitions
NPAD = 10240        # 80 blocks of 128
SHARD = NPAD // C   # 1280 nodes per core
BLK = SHARD // P    # 10 dst blocks per core
NBLK = NPAD // P    # 80 src blocks total
YCH = 512           # node-chunk width for the replicated y1 dense layer
F32 = mybir.dt.float32
BF16 = mybir.dt.bfloat16
import ml_dtypes

NPBF16 = ml_dtypes.bfloat16

_DEBUG_OUTPUTS = bool(int(os.environ.get("KERNEL_DEBUG_OUTPUTS", "0")))


# ------------------------------------------------------- tile drain patch
def _patched_drain_and_barrier(self, tick_clock, wait_clock):
    """walrus in this container rejects >1 sync-wait command on the tail
    Drain; spread the waits across SP nops (1 wait each)."""
    nc = self.nc
    drain_inst = nc.sync.drain()
    wait_clock.add_sem_waits(
        drain_inst.ins, ScopedClock({None: tick_clock.global_clock})
    )
    waits = list(drain_inst.ins.sync_info.on_wait)
    if len(waits) > 1:
        upd = list(drain_inst.ins.sync_info.on_update)
        drain_inst.ins.sync_info = mybir.SyncInfo(on_wait=waits[:1], on_update=upd)
        for i, w in enumerate(waits[1:]):
            nop = nc.sync.nop(nofuse=True, hint=f"tailwait{i}")
            nop.ins.sync_info = mybir.SyncInfo(on_wait=[w], on_update=[])
    nc.all_engine_barrier()
    assert self.sems is not None
    popped = nc._tile_sem_poison_stack.pop()
    assert popped is self._sem_poison
    nc.clear_and_free_semaphores(list(self.sems.allocated().values()))
    nc.all_engine_barrier()


tile.TileContext._drain_and_barrier = _patched_drain_and_barrier

_noop_ctr = [0]


def _split_excess_waits(nc, lim=1):
    """walrus in this container caps sync-wait commands per instruction;
    spill excess waits onto same-engine NOPs placed just before."""
    nsplit = 0
    for fn in nc.m.functions:
        for b in fn.blocks:
            newl = []
            changed = False
            for inst in b.instructions:
                si = inst.sync_info
                if si is not None and len(si.on_wait) > lim:
                    waits = list(si.on_wait)
                    head, tail = waits[: len(waits) - lim], waits[len(waits) - lim :]
                    for i in range(0, len(head), lim):
                        _noop_ctr[0] += 1
                        nop = mybir.InstNoOp(
                            name=f"waitnop-{_noop_ctr[0]}",
                            sync_info=mybir.SyncInfo(
                                on_wait=head[i : i + lim], on_update=[]
                            ),
                            bass_nofuse=True,
                            engine=inst.engine,
                        )
                        newl.append(nop)
                    inst.sync_info = mybir.SyncInfo(
                        on_wait=tail, on_update=list(si.on_update)
                    )
                    nsplit += 1
                    changed = True
                newl.append(inst)
            if changed:
                b.instructions = newl
    return nsplit


# ------------------------------------------------------- host preprocessing
def preprocess(edge_index, edge_weight):
    row = edge_index[0].astype(np.int64)
    col = edge_index[1].astype(np.int64)
    ew = edge_weight.astype(np.float32)

    deg = np.zeros(NPAD, np.float32)
    np.add.at(deg, col, ew)
    deg[:N] += 1.0  # self loops
    dis = np.zeros(NPAD, np.float32)
    nz = deg > 0
    dis[nz] = 1.0 / np.sqrt(deg[nz])
    sqdeg = np.sqrt(deg)

    # dense normalized operator M[s, d] = dis[s] * (A + I)[s, d] * dis[d]
    M = np.zeros((NPAD, NPAD), np.float32)
    np.add.at(M, (row, col), ew)
    idx = np.arange(N)
    M[idx, idx] += 1.0
    M *= dis[:, None]
    M *= dis[None, :]

    # per-core slab tables: mt[c][p, (bp*BLK + b)*P + q] = M[bp*P+p, c*SHARD+b*P+q]
    Mr = M.reshape(NBLK, P, C, BLK * P)
    mt = np.empty((C, P, NBLK * BLK * P), NPBF16)
    for c in range(C):
        mt[c] = (
            np.ascontiguousarray(Mr[:, :, c, :].transpose(1, 0, 2))
            .reshape(P, NBLK * BLK * P)
            .astype(NPBF16)
        )

    d_row = np.zeros(NPAD, np.float32)
    np.add.at(d_row, row, ew)

    mask = np.zeros(NPAD, np.float32)
    mask[:N] = 1.0

    def shard_cols(v):
        # [NPAD] -> [C, P, BLK]  ([p, b] = v[c*SHARD + b*P + p])
        return np.ascontiguousarray(v.reshape(C, BLK, P).transpose(0, 2, 1))

    return dict(
        mt=mt,
        d_row=shard_cols(d_row),
        sqdeg=shard_cols(sqdeg),
        mask=shard_cols(mask),
    )


# --------------------------------------------------------- device program
def build_program(for_sim=False):
    nc = bass.Bass(num_devices=C)
    dp = nc.declare_dram_parameter

    x_fl = dp("x_full", [NPAD, FIN], BF16, isOutput=False)
    mt = dp("mt", [P, NBLK * BLK * P], BF16, isOutput=False)
    w1 = dp("w1t", [P, 2 * P], BF16, isOutput=False)      # [fi, fo_c, fo]
    w2 = dp("w2t", [P, 2 * FH], BF16, isOutput=False)     # [fi_p, fi_c, fo]
    wp = dp("wpt", [P, 2 * K], F32, isOutput=False)       # [fo_p, fo_c, k]
    b1t = dp("b1t", [P, 2], F32, isOutput=False)
    b2r = dp("b2r", [1, FH], F32, isOutput=False)
    bpr = dp("bpr", [1, K], F32, isOutput=False)
    drow_t = dp("drow", [P, BLK], F32, isOutput=False)
    sqdeg_t = dp("sqdeg", [P, BLK], F32, isOutput=False)
    mask_t = dp("mask", [P, BLK], F32, isOutput=False)
    ident_t = dp("ident", [P, P], F32, isOutput=False)
    id64_t = dp("id64e", [K, K], F32, isOutput=False)  # I/sqrt(K)
    ones_t = dp("ones", [P, 1], F32, isOutput=False)
    ones_row_t = dp("ones_row", [1, P], F32, isOutput=False)

    out_t = dp("out", [1, 1], F32, isOutput=True)
    dbg = {}
    if _DEBUG_OUTPUTS:
        dbg["ag1"] = dp("dbg_ag1", [P, C * SHARD], BF16, isOutput=True)
        dbg["sh"] = dp("dbg_sh", [NPAD, K], BF16, isOutput=True)
        dbg["numden"] = dp("dbg_numden", [1, 2], F32, isOutput=True)
        dbg["ss"] = dp("dbg_ss", [K, K], F32, isOutput=True)

    # internal DRAM
    ag1_in = nc.dram_tensor("ag1_in", [P, SHARD], BF16)
    ag1_out = nc.dram_tensor("ag1_out", [C * P, SHARD], BF16, addr_space="Shared")
    sh_in = nc.dram_tensor("sh_in", [SHARD, K], BF16)
    sh_full = nc.dram_tensor("sh_full", [NPAD, K], BF16, addr_space="Shared")
    ar_in = nc.dram_tensor("ar_in", [K, K + 2], F32)
    ar_out = nc.dram_tensor("ar_out", [C * K, K + 2], F32, addr_space="Shared")

    rg = [list(range(C))]
    AG = lambda i, o: nc.gpsimd.collective_compute(
        "AllGather", mybir.AluOpType.bypass, replica_groups=rg, ins=[i], outs=[o]
    )

    with tile.TileContext(nc) as tc:
        with (
            tc.tile_pool(name="const", bufs=1) as cp,
            tc.tile_pool(name="big", bufs=1) as bigp,
            tc.tile_pool(name="mslab", bufs=3) as mp,
            tc.tile_pool(name="work", bufs=2) as wk,
            tc.tile_pool(name="acc", bufs=1) as accp,
            tc.tile_pool(name="ps", bufs=5, space="PSUM") as ps,
            tc.tile_pool(name="pss", bufs=1, space="PSUM") as pss,
            tc.tile_pool(name="psa", bufs=1, space="PSUM") as psa,
        ):
            def load(pool, name, src, shape, dtype=F32, eng=None):
                t = pool.tile(shape, dtype, tag=name)
                (eng or nc.sync).dma_start(out=t[:], in_=src)
                return t

            # x first on the ACT queue: it gates layer 1 while SP streams slabs.
            x_sb = load(
                bigp, "xsb", x_fl[:].rearrange("(b p) f -> p b f", p=P),
                [P, NBLK, FIN], BF16, eng=nc.scalar,
            )
            w1_sb = load(cp, "w1", w1[:].rearrange("p (c f) -> p c f", c=2),
                         [P, 2, P], BF16, eng=nc.scalar)
            w2_sb = load(cp, "w2", w2[:].rearrange("p (c f) -> p c f", c=2),
                         [P, 2, FH], BF16, eng=nc.scalar)
            wp_sb = load(cp, "wp", wp[:].rearrange("p (c f) -> p c f", c=2),
                         [P, 2, K], F32, eng=nc.scalar)
            b1_sb = load(cp, "b1t", b1t[:], [P, 2], F32, eng=nc.scalar)
            b2_sb = load(cp, "b2r", b2r[:], [1, FH], F32, eng=nc.scalar)
            bp_sb = load(cp, "bpr", bpr[:], [1, K], F32, eng=nc.scalar)
            drow_sb = load(cp, "drow", drow_t[:], [P, BLK], F32, eng=nc.scalar)
            sqdeg_sb = load(cp, "sqdeg", sqdeg_t[:], [P, BLK], F32, eng=nc.scalar)
            mask_sb = load(cp, "mask", mask_t[:], [P, BLK], F32, eng=nc.scalar)
            ident_sb = load(cp, "ident", ident_t[:], [P, P], F32, eng=nc.scalar)
            id64_sb = load(cp, "id64", id64_t[:], [K, K], F32, eng=nc.scalar)
            ones_sb = load(cp, "ones", ones_t[:], [P, 1], F32, eng=nc.scalar)
            ones_row_sb = load(cp, "ones_row", ones_row_t[:], [1, P], F32,
                               eng=nc.scalar)

            mt_dr = mt[:].rearrange("p (s b q) -> p s (b q)", s=NBLK // 2, q=P)

            import itertools
            _slabctr = itertools.count()

            def slab_load(g):
                # one transfer covers TWO src blocks (5KB/partition)
                slab = mp.tile(
                    [P, 2, BLK, P], BF16, tag="slab", name=f"slab{next(_slabctr)}"
                )
                nc.sync.dma_start(
                    out=slab[:],
                    in_=mt_dr[:, g, :].rearrange("p (t b q) -> p t b q", t=2, q=P),
                )
                return slab

            # ---------------- layer 1 aggregation: agg1T[fi, dst] = (M^T X)^T
            agg1t = [
                ps.tile([P, 4, P], F32, tag="acc2k", name=f"agg1_{i}")
                for i in range(3)
            ]
            # groups of dst blocks per psum bank: one wide matmul per group
            GRP = [(0, 4), (4, 8), (8, 10)]
            for g in range(NBLK // 2):
                slab = slab_load(g)
                for t in range(2):
                    bp_ = 2 * g + t
                    for i, (b0, b1) in enumerate(GRP):
                        nc.tensor.matmul(
                            agg1t[i][:, 0 : b1 - b0, :],
                            x_sb[:, bp_, :],
                            slab[:, t, b0:b1, :],
                            start=(bp_ == 0),
                            stop=(bp_ == NBLK - 1),
                        )
            a1sb = wk.tile([P, BLK, P], BF16, tag="a1sb")
            for i, (b0, b1) in enumerate(GRP):
                nc.vector.tensor_copy(
                    a1sb[:, b0:b1, :], agg1t[i][:, 0 : b1 - b0, :]
                )
            nc.sync.dma_start(
                out=ag1_in[:].rearrange("p (b q) -> p b q", q=P), in_=a1sb[:]
            )
            AG(ag1_in[:], ag1_out[:])

            # readback feature-major full aggregate [fi, node]
            a1T = bigp.tile([P, C * SHARD], BF16, tag="a1T")
            # AG-dependent readbacks go on the ACT queue so the SP queue's
            # slab streaming is not head-of-line blocked behind the AG wait.
            nc.scalar.dma_start(
                out=a1T[:].rearrange("f (c n) -> f c n", c=C),
                in_=ag1_out[:].rearrange("(c f) n -> f c n", f=P),
            )
            if _DEBUG_OUTPUTS:
                nc.scalar.dma_start(out=dbg["ag1"][:], in_=a1T[:])

            # ---------------- y1T = relu(W1^T agg1T + b1T)   [fo, node], bf16
            y1T = bigp.tile([P, 2, NPAD], BF16, tag="y1T")
            for fo_c in range(2):
                for ch in range(NPAD // YCH):
                    py = ps.tile([P, YCH], F32, tag="acc2k", name=f"py_{fo_c}_{ch}")
                    nc.tensor.matmul(
                        py[:], w1_sb[:, fo_c, :],
                        a1T[:, ch * YCH : (ch + 1) * YCH],
                        start=True, stop=True,
                    )
                    nc.scalar.activation(
                        y1T[:, fo_c, ch * YCH : (ch + 1) * YCH], py[:],
                        mybir.ActivationFunctionType.Relu,
                        bias=b1_sb[:, fo_c : fo_c + 1],
                    )

            # ---------------- layer 2: z = y1 @ W2 computed just-in-time per
            # src block, aggregated into h2[dst, fo] (node-major)
            h2t = [
                ps.tile([P, 2, FH], F32, tag="acc2k", name=f"h2_{i}")
                for i in range(5)
            ]
            h2 = [h2t[b // 2][:, b % 2, :] for b in range(BLK)]
            for g in range(NBLK // 2):
                slab = slab_load(g)
                for t in range(2):
                    bp_ = 2 * g + t
                    pz = pss.tile([P, FH], F32, tag="tr", name=f"pz_{bp_}")
                    for fi_c in range(2):
                        nc.tensor.matmul(
                            pz[:], y1T[:, fi_c, bp_ * P : (bp_ + 1) * P],
                            w2_sb[:, fi_c, :],
                            start=(fi_c == 0), stop=(fi_c == 1),
                        )
                    zt = wk.tile([P, FH], BF16, tag="zt")
                    nc.vector.tensor_copy(zt[:], pz[:])
                    for b in range(BLK):
                        nc.tensor.matmul(
                            h2[b], slab[:, t, b, :], zt[:],
                            start=(bp_ == 0 and b % 2 == 0), stop=False,
                        )
            for b in range(BLK):
                nc.tensor.matmul(
                    h2[b], ones_row_sb[:], b2_sb[:], start=False, stop=True
                )

            # ---------------- s = softmax(relu(h2) @ Wp + bp) per block
            s_sb = accp.tile([P, BLK, K], F32, tag="s")
            sh_sb = accp.tile([P, BLK, K], F32, tag="sh")
            shb_sb = accp.tile([P, BLK, K], BF16, tag="shb")
            ssq_sb = accp.tile([P, BLK], F32, tag="ssq")
            sscratch = wk.tile([P, K], F32, tag="sscratch")
            for b in range(BLK):
                o2 = wk.tile([P, FH], F32, tag="o2")
                nc.scalar.activation(
                    o2[:], h2[b], mybir.ActivationFunctionType.Relu
                )
                sp = pss.tile([P, K], F32, tag="sp")
                for c_ in range(2):
                    tr = pss.tile([P, P], F32, tag="tr")
                    nc.tensor.transpose(
                        tr[:], o2[:, c_ * P : (c_ + 1) * P], ident_sb[:]
                    )
                    tr_sb = wk.tile([P, P], F32, tag="tr_sb")
                    nc.vector.tensor_copy(tr_sb[:], tr[:])
                    nc.tensor.matmul(
                        sp[:], tr_sb[:], wp_sb[:, c_, :], start=(c_ == 0),
                        stop=False,
                    )
                nc.tensor.matmul(
                    sp[:], ones_row_sb[:], bp_sb[:], start=False, stop=True
                )
                smax = wk.tile([P, 1], F32, tag="smax")
                nc.vector.tensor_reduce(
                    smax[:], sp[:], axis=mybir.AxisListType.X,
                    op=mybir.AluOpType.max, negate=True,
                )
                sexp = wk.tile([P, K], F32, tag="sexp")
                ssum = wk.tile([P, 1], F32, tag="ssum")
                nc.scalar.activation(
                    sexp[:], sp[:], mybir.ActivationFunctionType.Exp,
                    bias=smax[:], accum_out=ssum[:],
                )
                nc.vector.reciprocal(ssum[:], ssum[:])
                nc.vector.tensor_scalar(
                    s_sb[:, b, :], sexp[:], ssum[:], mask_sb[:, b : b + 1],
                    op0=mybir.AluOpType.mult, op1=mybir.AluOpType.mult,
                )
                nc.scalar.activation(
                    sscratch[:], s_sb[:, b, :],
                    mybir.ActivationFunctionType.Square,
                    accum_out=ssq_sb[:, b : b + 1],
                )
                nc.vector.tensor_scalar_mul(
                    sh_sb[:, b, :], s_sb[:, b, :], sqdeg_sb[:, b : b + 1]
                )
                nc.vector.tensor_copy(shb_sb[:, b, :], sh_sb[:, b, :])
            nc.sync.dma_start(
                out=sh_in[:].rearrange("(b p) k -> p b k", p=P), in_=shb_sb[:]
            )
            AG(sh_in[:], sh_full[:])
            if _DEBUG_OUTPUTS:
                nc.scalar.dma_start(out=dbg["sh"][:], in_=sh_full[:])

            # ss = S^T S partial (local shard) — overlaps the AllGather
            psbig = psa.tile([P, K + 8], F32, tag="psbig")
            ss_psum = psbig[0:K, 0:K]
            smalls = psbig[:, K : K + 8]
            for b in range(BLK):
                nc.tensor.matmul(
                    ss_psum, s_sb[:, b, :], s_sb[:, b, :],
                    start=(b == 0), stop=(b == BLK - 1),
                )

            # ---------------- pool aggregation: hp[dst, k] = aggM(shat)
            shf = bigp.tile([P, NBLK, K], BF16, tag="shf")
            nc.scalar.dma_start(
                out=shf[:], in_=sh_full[:].rearrange("(b p) k -> p b k", p=P)
            )
            # transposed pool output [k, dst] so each matmul covers a 4-block
            # group (N=512); shat^T tiles for the elementwise num reduction
            shT = wk.tile([K, BLK, P], F32, tag="shT")
            for b in range(BLK):
                trs = pss.tile([K, P], F32, tag="tr", name=f"trs_{b}")
                nc.tensor.transpose(trs[:], sh_sb[:, b, :], ident_sb[:])
                nc.vector.tensor_copy(shT[:, b, :], trs[:])
            hpt = [
                ps.tile([K, 4 * P], F32, tag="acc2k", name=f"hp_{i}")
                for i in range(3)
            ]
            for g in range(NBLK // 2):
                slab = slab_load(g)
                for t in range(2):
                    bp_ = 2 * g + t
                    for i, (b0, b1) in enumerate(GRP):
                        nc.tensor.matmul(
                            hpt[i][:, 0 : (b1 - b0) * P],
                            shf[:, bp_, :],
                            slab[:, t, b0:b1, :],
                            start=(bp_ == 0),
                            stop=(bp_ == NBLK - 1),
                        )
            num_sb = accp.tile([K, BLK], F32, tag="num")
            nscratch = wk.tile([K, 4 * P], F32, tag="nscratch")
            for i, (b0, b1) in enumerate(GRP):
                w_ = (b1 - b0) * P
                nc.vector.tensor_tensor(
                    out=nscratch[:, 0:w_],
                    in0=shT[:, b0:b1, :].reshape([K, w_]),
                    in1=hpt[i][:, 0:w_],
                    op=mybir.AluOpType.mult,
                )
                nc.vector.tensor_reduce(
                    num_sb[:, i : i + 1], nscratch[:, 0:w_],
                    axis=mybir.AxisListType.X, op=mybir.AluOpType.add,
                )

            # ---------------- packed partial reduce: [ss | num | den]
            red = wk.tile([P, 1], F32, tag="red")
            nc.vector.tensor_reduce(
                red[:], num_sb[:], axis=mybir.AxisListType.X,
                op=mybir.AluOpType.add,
            )
            redq = wk.tile([P, 1], F32, tag="redq")
            nc.vector.tensor_reduce(
                redq[:], ssq_sb[:], axis=mybir.AxisListType.X,
                op=mybir.AluOpType.add,
            )
            # num_partial = sum(shat*aggM(shat)) - sum(s*s)
            nc.vector.tensor_tensor(
                out=red[:], in0=red[:], in1=redq[:], op=mybir.AluOpType.subtract
            )
            num_ps = smalls[0:1, 0:1]
            nc.tensor.matmul(num_ps, red[:], ones_sb[:], start=True, stop=True)
            den_sb = wk.tile([P, BLK], F32, tag="den")
            nc.vector.tensor_tensor(
                out=den_sb[:], in0=ssq_sb[:], in1=drow_sb[:],
                op=mybir.AluOpType.mult,
            )
            red2 = wk.tile([P, 1], F32, tag="red2")
            nc.vector.tensor_reduce(
                red2[:], den_sb[:], axis=mybir.AxisListType.X,
                op=mybir.AluOpType.add,
            )
            den_ps = smalls[0:1, 1:2]
            nc.tensor.matmul(den_ps, red2[:], ones_sb[:], start=True, stop=True)

            arbuf = wk.tile([K, K + 2], F32, tag="arbuf")
            nc.vector.memset(arbuf[:], 0.0)
            nc.vector.tensor_copy(arbuf[:, 0:K], ss_psum)
            nc.vector.tensor_copy(arbuf[0:1, K : K + 1], num_ps)
            nc.vector.tensor_copy(arbuf[0:1, K + 1 : K + 2], den_ps)
            nc.sync.dma_start(out=ar_in[:], in_=arbuf[:])
            AG(ar_in[:], ar_out[:])
            gath = wk.tile([K, C, K + 2], F32, tag="gath")
            nc.scalar.dma_start(
                out=gath[:], in_=ar_out[:].rearrange("(c r) f -> r c f", r=K)
            )
            acc = wk.tile([K, K + 2], F32, tag="acc")
            nc.vector.tensor_copy(acc[:], gath[:, 0, :])
            for c_ in range(1, C):
                nc.vector.tensor_tensor(
                    out=acc[:], in0=acc[:], in1=gath[:, c_, :],
                    op=mybir.AluOpType.add,
                )
            ss_sb = acc[:, 0:K]
            ndg_sb = acc[0:1, K : K + 2]
            if _DEBUG_OUTPUTS:
                nc.sync.dma_start(out=dbg["ss"][:], in_=ss_sb)
                nc.sync.dma_start(out=dbg["numden"][:], in_=ndg_sb)

            # ---------------- ortho loss + final scalar
            sq64 = wk.tile([K, K], F32, tag="sq64")
            col64 = wk.tile([K, 1], F32, tag="col64")
            nc.scalar.activation(
                sq64[:], ss_sb, mybir.ActivationFunctionType.Square,
                accum_out=col64[:],
            )
            fro_ps = smalls[0:1, 2:3]
            nc.tensor.matmul(fro_ps, col64[:], ones_sb[:K, :], start=True, stop=True)
            fro = wk.tile([1, 1], F32, tag="fro_sb")
            nc.scalar.sqrt(fro[:], fro_ps)
            nc.vector.reciprocal(fro[:], fro[:])
            # broadcast 1/fro to K partitions via rank-1 matmul
            fro_bc = smalls[0:K, 3:4]
            nc.tensor.matmul(
                fro_bc, ones_row_sb[:, :K], fro[:], start=True, stop=True
            )
            fro64 = wk.tile([K, 1], F32, tag="fro64")
            nc.vector.tensor_copy(fro64[:], fro_bc)
            # t = ss/fro - I/sqrt(K)
            tmat = wk.tile([K, K], F32, tag="tmat")
            nc.vector.tensor_scalar_mul(tmat[:], ss_sb, fro64[:])
            nc.vector.tensor_tensor(
                out=tmat[:], in0=tmat[:], in1=id64_sb[:],
                op=mybir.AluOpType.subtract,
            )
            nc.scalar.activation(
                sq64[:], tmat[:], mybir.ActivationFunctionType.Square,
                accum_out=col64[:],
            )
            orth_ps = smalls[0:1, 4:5]
            nc.tensor.matmul(orth_ps, col64[:], ones_sb[:K, :], start=True, stop=True)
            orth = wk.tile([1, 1], F32, tag="orth_sb")
            nc.scalar.sqrt(orth[:], orth_ps)

            rden = wk.tile([1, 1], F32, tag="rden")
            nc.vector.reciprocal(rden[:], acc[0:1, K + 1 : K + 2])
            mloss = wk.tile([1, 1], F32, tag="mloss")
            nc.vector.tensor_tensor(
                out=mloss[:], in0=acc[0:1, K : K + 1], in1=rden[:],
                op=mybir.AluOpType.mult,
            )
            res = wk.tile([1, 1], F32, tag="res")
            nc.vector.tensor_tensor(
                out=res[:], in0=orth[:], in1=mloss[:], op=mybir.AluOpType.subtract
            )
            nc.sync.dma_start(out=out_t[:], in_=res[:])

    if not for_sim:
        _split_excess_waits(nc)
    lower_extended_insts(nc)
    return nc


_PROG_CACHE = {}


def _get_program(key=()):
    if key not in _PROG_CACHE:
        _PROG_CACHE[key] = build_program()
    return _PROG_CACHE[key]


def make_in_maps(inputs, tabs):
    x = np.asarray(inputs["x"], np.float32)
    W1, W2, Wp = (np.asarray(inputs[k], np.float32) for k in ("W1", "W2", "Wp"))
    b1, b2, bp = (np.asarray(inputs[k], np.float32) for k in ("b1", "b2", "bp"))
    xpad = np.zeros((NPAD, FIN), np.float32)
    xpad[:N] = x
    ident = np.eye(P, dtype=np.float32)
    id64e = (np.eye(K, dtype=np.float32) / np.sqrt(np.float32(K))).astype(np.float32)

    common = dict(
        x_full=xpad.astype(NPBF16),
        w1t=W1.reshape(P, 2 * P).astype(NPBF16),
        w2t=W2.reshape(2, P, FH).transpose(1, 0, 2).reshape(P, 2 * FH).astype(NPBF16),
        wpt=Wp.reshape(2, P, K).transpose(1, 0, 2).reshape(P, 2 * K).astype(np.float32),
        b1t=np.ascontiguousarray(b1.reshape(2, P).T),
        b2r=b2.reshape(1, FH),
        bpr=bp.reshape(1, K),
        ident=ident,
        id64e=id64e,
        ones=np.ones((P, 1), np.float32),
        ones_row=np.ones((1, P), np.float32),
    )
    in_maps = []
    for c in range(C):
        in_maps.append(
            dict(
                common,
                mt=tabs["mt"][c],
                drow=tabs["d_row"][c],
                sqdeg=tabs["sqdeg"][c],
                mask=tabs["mask"][c],
            )
        )
    return in_maps


def kernel(x, edge_index, edge_weight, W1, b1, W2, b2, Wp, bp):
    edge_index = np.asarray(edge_index)
    edge_weight = np.asarray(edge_weight, np.float32)
    tabs = preprocess(edge_index, edge_weight)
    nc = _get_program()
    in_maps = make_in_maps(
        dict(x=x, W1=W1, b1=b1, W2=W2, b2=b2, Wp=Wp, bp=bp), tabs
    )
    trace = bool(int(os.environ.get("KERNEL_TRACE", "0")))
    kwargs = {}
    if trace:
        kwargs = dict(trace=True, tmpdir=os.environ.get("KERNEL_TRACE_DIR"))
    res = run_bass_kernel_spmd(nc, in_maps, core_ids=list(range(C)), **kwargs)
    if trace:
        kernel.exec_time_ns = res.exec_time_ns
        kernel.mean_exec_time_ns = res.mean_exec_time_ns
        kernel.bass_results = res
    out = res.results[0]["out"].reshape(())
    if _DEBUG_OUTPUTS:
        kernel.debug = {
            k: res.results[0][f"dbg_{k}"] for k in ("ag1", "sh", "numden", "ss")
        }
    return np.float32(out)


if __name__ == "__main__":
    import reference

    inputs = reference.setup_inputs()
    inputs = {k: np.asarray(v) for k, v in inputs.items()}
    got = kernel(**inputs)
    print("kernel out:", got)
